# revision 61
# baseline (speedup 1.0000x reference)
"""GCN mean-aggregation + linear on 8 Trainium2 NeuronCores.

out = (segment_sum(x[col], row) / deg(row)) @ W.T + b

Strategy:
  - Destinations (rows of the output) are sharded across 8 cores, 6250 each.
  - Sources are split into lo/hi halves of 25000 rows so gather indices fit
    int16 (dma_gather sign-extends its indices).
  - x is cast to bf16 on the host; each core bulk-gathers the 256-byte
    feature rows of its edges with dma_gather (edge-on-partition layout).
  - Segment-sum is done on the TensorEngine: for each 128-edge tile,
    psum[feat, c] += Y.T @ S where S is a streamed one-hot [128, 32] that
    maps each edge slot to its destination column.  Tile t owns the 16
    destinations [16t, 16t+16) and can also address the next 16 (window 32),
    which lets an overflowing tile push excess edges one tile backward.
  - Edges that still don't fit (~2.4%) are fixed up on the host.
  - The final 128x128 linear, degree division and bias run on-chip.
"""

import os
import sys

import numpy as np
import ml_dtypes

# ---- static problem configuration (hardcoded per the task contract) ----
N_NODES = 50000
N_EDGES = 800000
D = 128
NC = 8
SHARD = N_NODES // NC          # 6250 destinations per core
HALF = N_NODES // 2            # 25000 source rows per gather table
V_T = 16                       # destinations owned per 128-slot tile
M_WIN = 32                     # S window columns (own 16 + next 16)
R_TILES = 32                   # tiles per round: 32*32 = 1024 psum cols (2 banks)
TILES_HALF = -(-SHARD // V_T)  # 417
N_TILES = 2 * TILES_HALF       # 834
NBLK = -(-SHARD // 128)        # 49 output row blocks
PSUM_BANK = 512                # f32 columns per PSUM bank

for _p in ("/opt/trn_rl_repo", "/root/.axon_site/_ro/trn_rl_repo"):
    if os.path.isdir(_p) and _p not in sys.path:
        sys.path.insert(0, _p)


# --------------------------------------------------------------------------
# Host-side edge packing
# --------------------------------------------------------------------------

def _pack_half(vd, src, shard, v_t, m_win, tiles_half):
    """Pack one (core, half)'s edges into fixed 128-slot tiles.

    vd:  local destination id in [0, shard) per edge
    src: gather index in [0, HALF) per edge

    Returns (slot_idx[tiles*128] int16, slot_col[tiles*128] int8 (-1 = pad),
             spill_mask over input edges).
    """
    n = len(vd)
    order = np.argsort(vd, kind="stable")
    vd_s = vd[order]
    src_s = src[order]
    cnt = np.bincount(vd_s, minlength=shard)
    # per-tile own edge ranges in the sorted arrays
    tile_cnt = np.add.reduceat(cnt, np.arange(0, shard, v_t)) if shard else np.zeros(0, np.int64)
    tile_end = np.cumsum(tile_cnt)
    tile_start = tile_end - tile_cnt

    slot_idx = np.zeros(tiles_half * 128, dtype=np.int16)
    slot_col = np.full(tiles_half * 128, -1, dtype=np.int8)
    spill_sorted = np.zeros(n, dtype=bool)

    kept = np.minimum(tile_cnt, 128)
    moved = np.zeros(tiles_half, dtype=np.int64)   # edges moved from t into t-1
    spill_ct = np.zeros(tiles_half, dtype=np.int64)
    for t in range(tiles_half):
        excess = tile_cnt[t] - 128
        if excess > 0:
            if t > 0:
                # tile t-1 receives edges only from t, so its free space
                # when t is processed is just 128 - kept[t-1]
                moved[t] = min(excess, max(0, 128 - kept[t - 1]))
            spill_ct[t] = excess - moved[t]

    for t in range(tiles_half):
        s, e = tile_start[t], tile_end[t]
        own_keep = kept[t]  # first own_keep own edges stay
        # own kept edges -> slots [0, own_keep)
        sl = t * 128
        if own_keep > 0:
            sel = slice(s, s + own_keep)
            slot_idx[sl:sl + own_keep] = src_s[sel]
            slot_col[sl:sl + own_keep] = (vd_s[sel] - t * v_t).astype(np.int8)
        pos = own_keep
        # moved-in edges from tile t+1
        if t + 1 < tiles_half and moved[t + 1] > 0:
            m = moved[t + 1]
            s2 = tile_start[t + 1] + kept[t + 1]
            sel = slice(s2, s2 + m)
            slot_idx[sl + pos:sl + pos + m] = src_s[sel]
            cols = vd_s[sel] - t * v_t
            assert cols.min() >= 0 and cols.max() < m_win
            slot_col[sl + pos:sl + pos + m] = cols.astype(np.int8)
            pos += m
        # spilled edges of tile t -> host
        if spill_ct[t] > 0:
            s3 = tile_start[t] + kept[t] + moved[t]
            spill_sorted[s3:s3 + spill_ct[t]] = True

    spill_mask = np.zeros(n, dtype=bool)
    spill_mask[order] = spill_sorted
    return slot_idx, slot_col, spill_mask


def _pack_inputs(x, edge_index, W, b):
    """Build per-core numpy input dicts + host spill info."""
    x = np.asarray(x, dtype=np.float32)
    ei = np.asarray(edge_index)
    W = np.asarray(W, dtype=np.float32)
    b = np.asarray(b, dtype=np.float32)
    row = ei[0].astype(np.int64)
    col = ei[1].astype(np.int64)

    x_bf = x.astype(ml_dtypes.bfloat16)
    xlo = np.ascontiguousarray(x_bf[:HALF])
    xhi = np.ascontiguousarray(x_bf[HALF:])
    wt = np.ascontiguousarray(W.T)                       # [din, dout]
    bb = np.ascontiguousarray(np.tile(b[None, :], (128, 1)))

    deg = np.bincount(row, minlength=N_NODES).astype(np.float64)
    with np.errstate(divide="ignore"):
        recip_full = np.where(deg > 0, 1.0 / deg, np.inf).astype(np.float32)

    core = row // SHARD
    in_maps = []
    spill_rows = []
    spill_cols = []
    for k in range(NC):
        sel = core == k
        r = row[sel] - k * SHARD
        c = col[sel]
        lo = c < HALF

        idx_all = np.zeros(N_TILES * 128, dtype=np.int16)
        col_all = np.full(N_TILES * 128, -1, dtype=np.int8)
        for h in (0, 1):
            hm = lo if h == 0 else ~lo
            vd = r[hm]
            src = (c[hm] - h * HALF).astype(np.int64)
            si, sc, sp = _pack_half(vd, src, SHARD, V_T, M_WIN, TILES_HALF)
            o = h * TILES_HALF * 128
            idx_all[o:o + TILES_HALF * 128] = si
            col_all[o:o + TILES_HALF * 128] = sc
            if sp.any():
                spill_rows.append(np.nonzero(sel)[0][hm][sp])

        # S one-hot stream: [128, N_TILES * M_WIN] bf16
        smat = np.zeros((128, N_TILES * M_WIN), dtype=ml_dtypes.bfloat16)
        slots = np.nonzero(col_all >= 0)[0]
        t_of = slots // 128
        p_of = slots % 128
        smat[p_of, t_of * M_WIN + col_all[slots]] = 1.0

        # gather index layout: global slot i -> [i % 16, i // 16], and the
        # 16-partition pattern replicated to all 128 partitions (the Q7
        # desc-gen cores each read their own 16-partition stripe)
        idx16 = np.ascontiguousarray(
            np.tile(idx_all.reshape(-1, 16).T, (8, 1)))

        rc = recip_full[k * SHARD:(k + 1) * SHARD]
        rc_pad = np.zeros(NBLK * 128, dtype=np.float32)
        rc_pad[:SHARD] = rc
        recip = np.ascontiguousarray(rc_pad.reshape(NBLK, 128).T)

        in_maps.append({
            "xlo": xlo, "xhi": xhi,
            "idx": idx16, "smat": smat,
            "recip": recip, "wt": wt, "bb": bb,
        })

    if spill_rows:
        sidx = np.concatenate(spill_rows)
    else:
        sidx = np.zeros(0, dtype=np.int64)
    return in_maps, row[sidx] if len(sidx) else np.zeros(0, np.int64), \
        col[sidx] if len(sidx) else np.zeros(0, np.int64), recip_full, W, b


# --------------------------------------------------------------------------
# Device program
# --------------------------------------------------------------------------

def _build_nc():
    # full region-overlap analysis so a covering writer resets a tile's
    # dependency set (keeps the gather's sem-wait count within ISA limits)
    os.environ["TILE_EXHAUSTIVE_MEMORY_SHARE_CHECK"] = "1"
    import concourse.bacc as bacc
    import concourse.mybir as mybir
    import concourse.tile as tile
    from concourse import library_config

    dt = mybir.dt
    # enlarged SWDGE descriptor-ring carveout: one 6144-index gather emits
    # ~24.6KB of descriptors per ring partition, so the default 16KB ring
    # makes the Q7 descriptor generator stall on ring space mid-gather
    # (57.8us vs 43.7us per gather measured; 48KB was worse than 32KB)
    nc = bacc.Bacc(None, target_bir_lowering=False, debug=False,
                   dynamic_dma_scratch_size=32768, num_swdge_queues=2)

    xlo = nc.dram_tensor("xlo", [HALF, D], dt.bfloat16, kind="ExternalInput").ap()
    xhi = nc.dram_tensor("xhi", [HALF, D], dt.bfloat16, kind="ExternalInput").ap()
    idx = nc.dram_tensor("idx", [128, N_TILES * 8], dt.int16,
                         kind="ExternalInput").ap()
    smat = nc.dram_tensor("smat", [128, N_TILES * M_WIN], dt.bfloat16,
                          kind="ExternalInput").ap()
    recip = nc.dram_tensor("recip", [128, NBLK], dt.float32, kind="ExternalInput").ap()
    wt = nc.dram_tensor("wt", [D, D], dt.float32, kind="ExternalInput").ap()
    bb = nc.dram_tensor("bb", [128, D], dt.float32, kind="ExternalInput").ap()
    # padded to a whole number of 128-row blocks so one uniform-stride DMA
    # covers everything; the host slices off the pad rows
    out = nc.dram_tensor("out", [NBLK * 128, D], dt.float32,
                         kind="ExternalOutput").ap()

    with tile.TileContext(nc) as tc:
        with (
            tc.tile_pool(name="const", bufs=1) as constp,
            tc.tile_pool(name="agg", bufs=1) as aggp,
            tc.tile_pool(name="io", bufs=6) as iop,
            tc.tile_pool(name="fin", bufs=1) as finp,
            tc.tile_pool(name="psum", bufs=2, space="PSUM") as psp,
            tc.tile_pool(name="psum2", bufs=2, space="PSUM") as ps2p,
        ):
            wt_sb = constp.tile([D, D], dt.float32)
            wt_dma = nc.sync.dma_start(out=wt_sb[:], in_=wt[:, :])
            bb_sb = constp.tile([128, D], dt.float32)
            bb_dma = nc.sync.dma_start(out=bb_sb[:], in_=bb[:, :])
            recip_sb = constp.tile([128, NBLK], dt.float32)
            recip_dma = nc.sync.dma_start(out=recip_sb[:], in_=recip[:, :])
            # idx and S live in SBUF for the whole kernel: loading them
            # once keeps the inner loop free of HWDGE DMAs (whose pseudo
            # instruction supports only a single semaphore wait).
            idx_sb = constp.tile([128, N_TILES * 8], dt.int16)
            idx_dma = nc.sync.dma_start(out=idx_sb[:], in_=idx[:, :])
            smat_sb = constp.tile([128, N_TILES * M_WIN], dt.bfloat16)
            smat_dma = nc.sync.dma_start(out=smat_sb[:], in_=smat[:, :])

            aggT = aggp.tile([128, 2 * SHARD + M_WIN], dt.float32)
            nc.vector.memset(aggT[:], 0.0)
            # allocated up-front: sharing SBUF with released y-tiles would
            # add pool-overlap waits onto the PE Ldweights of the final
            # linear, which (hw-decoded) supports only a single sem wait
            o_all = constp.tile([128, NBLK * D], dt.float32)

            # per-psum/y-buffer-parity records of the previous generation
            prev_use = {}     # parity -> (gather, last matmul)
            prev_evict = {}   # parity -> last eviction
            gathers = []      # all gather instructions, in order
            ridx = 0
            for h in (0, 1):
                xsrc = xlo if h == 0 else xhi
                for r0 in range(0, TILES_HALF, R_TILES):
                    nt = min(R_TILES, TILES_HALF - r0)
                    g0 = h * TILES_HALF + r0
                    slots = nt * 128
                    par = ridx % 2

                    y_sb = iop.tile([128, nt * D], dt.bfloat16, tag="y")
                    y3 = y_sb[:].rearrange("p (t e) -> p t e", e=D)
                    # single_packet packs all descriptors into one SDMA
                    # packet, whose 64-descriptor/lane ceiling caps a gather
                    # at 1024 indices — larger gathers crash the exec unit.
                    # queue_num must be slot-sticky (ridx%2 with 6 y bufs):
                    # same-buffer gathers on one FIFO keeps their WAW safe
                    g = nc.gpsimd.dma_gather(
                        y3, xsrc, idx_sb[:, g0 * 8:(g0 + nt) * 8],
                        slots, slots, D, elem_step=D, single_packet=False,
                        queue_num=ridx % 2)
                    gathers.append(g.ins)

                    ps = psp.tile([128, R_TILES * M_WIN], dt.float32)
                    first_mm = last_mm = None
                    for tl in range(nt):
                        c0 = tl * M_WIN
                        s0 = (g0 + tl) * M_WIN
                        # split at PSUM bank boundaries
                        cuts = [c0]
                        nb = (c0 // PSUM_BANK + 1) * PSUM_BANK
                        while nb < c0 + M_WIN:
                            cuts.append(nb)
                            nb += PSUM_BANK
                        cuts.append(c0 + M_WIN)
                        for a, bnd in zip(cuts[:-1], cuts[1:]):
                            last_mm = nc.tensor.matmul(
                                out=ps[:, a:bnd],
                                lhsT=y3[:, tl, :],
                                rhs=smat_sb[:, s0 + (a - c0):s0 + (bnd - c0)],
                                start=True, stop=True,
                            )
                            if first_mm is None:
                                first_mm = last_mm
                    prev_use[par] = (g.ins, last_mm.ins)

                    # evict with parity-split strided adds
                    base = h * SHARD + r0 * V_T
                    ps3 = ps[:, :nt * M_WIN].rearrange("p (t c) -> p t c", c=M_WIN)
                    n_even = (nt + 1) // 2
                    n_odd = nt // 2
                    dst_e = aggT[:, base:base + n_even * M_WIN].rearrange(
                        "p (t c) -> p t c", c=M_WIN)
                    ev = nc.vector.tensor_add(
                        out=dst_e, in0=dst_e, in1=ps3[:, 0::2, :])
                    if n_odd:
                        dst_o = aggT[:, base + V_T:
                                     base + V_T + n_odd * M_WIN].rearrange(
                            "p (t c) -> p t c", c=M_WIN)
                        ev = nc.vector.tensor_add(
                            out=dst_o, in0=dst_o, in1=ps3[:, 1::2, :])
                    prev_evict[par] = ev.ins
                    ridx += 1

            # combine lo+hi halves
            comb = nc.vector.tensor_add(
                out=aggT[:, :SHARD],
                in0=aggT[:, :SHARD],
                in1=aggT[:, SHARD:2 * SHARD],
            )

            # final linear + degree scale + bias, staged fully in SBUF so a
            # single DMA (single sem wait) writes the output
            nc.vector.memset(o_all[:], 0.0)
            first = True
            for blk in range(NBLK):
                rows = min(128, SHARD - blk * 128)
                ps2 = ps2p.tile([128, D], dt.float32)
                mm2 = nc.tensor.matmul(
                    out=ps2[:rows, :],
                    lhsT=aggT[:, blk * 128:blk * 128 + rows],
                    rhs=wt_sb[:],
                    start=True, stop=True,
                )
                osl = o_all[:, blk * D:(blk + 1) * D]
                ts = nc.vector.tensor_scalar_mul(
                    osl[:rows, :], ps2[:rows, :], recip_sb[:rows, blk:blk + 1])
                last_bias = nc.vector.tensor_add(
                    out=osl[:rows, :], in0=osl[:rows, :], in1=bb_sb[:rows, :])
            out_dma = nc.sync.dma_start(
                out=out.rearrange("(b p) j -> p b j", p=128),
                in_=o_all[:].rearrange("p (b j) -> p b j", j=D),
            )

    # Bacc.compile splits multi-sem waits into EventSemaphore instructions
    # (the walrus ISA allows only one wait per instruction), inserts
    # library reloads, and fills in extended-ISA instruction bytes.
    nc.compile()
    return nc


# --------------------------------------------------------------------------
# Entry point
# --------------------------------------------------------------------------

def kernel(x, edge_index, W, b, _want_profile=False):
    from concourse.bass_utils import run_bass_kernel_spmd

    in_maps, sp_row, sp_col, recip_full, W_np, b_np = _pack_inputs(
        x, edge_index, W, b)

    nc = _build_nc()
    res = run_bass_kernel_spmd(nc, in_maps, list(range(NC)),
                               trace=_want_profile)

    out_full = np.empty((N_NODES, D), dtype=np.float32)
    for k in range(NC):
        out_full[k * SHARD:(k + 1) * SHARD] = res.results[k]["out"][:SHARD]

    # host fixup for spilled edges: out[r] += (x[c] @ W.T) / deg[r]
    if len(sp_row):
        xs = np.asarray(x, dtype=np.float32)[sp_col]
        contrib = xs @ W_np.T
        order = np.argsort(sp_row, kind="stable")
        r_s = sp_row[order]
        c_s = contrib[order] * recip_full[r_s][:, None]
        bounds = np.nonzero(np.diff(r_s))[0] + 1
        starts = np.concatenate(([0], bounds))
        sums = np.add.reduceat(c_s, starts, axis=0)
        out_full[r_s[starts]] += sums

    if _want_profile:
        return out_full, res
    return out_full


if __name__ == "__main__":
    # quick self-exercise with random data
    rng = np.random.default_rng(0)
    x = rng.standard_normal((N_NODES, D), dtype=np.float32)
    ei = rng.integers(0, N_NODES, size=(2, N_EDGES)).astype(np.int64)
    W = rng.standard_normal((D, D), dtype=np.float32) / np.sqrt(D)
    b = rng.standard_normal(D, dtype=np.float32) * 0.01
    out = kernel(x, ei, W, b)
    print("out", out.shape, out.dtype)
